# revision 16
# baseline (speedup 1.0000x reference)
"""Trainium2 Bass kernel for GNN aggregate-update (scatter-mean + concat + MLP).

Strategy (8 NeuronCores, SPMD, no collectives):
  - Host (sharding/routing only): sort edge ids by target node and route each
    edge's feature row to the core that owns its target (cores own contiguous
    1/8 node ranges). Each core's edges land in one contiguous bf16 buffer,
    grouped by 64-node block; all 8 blocks of a 512-node MLP group share one
    padded per-block capacity so the whole group loads with a single DMA.
  - Device, per core: per group, ONE strided DMA (alternating between the two
    HWDGE rings) loads 8 blocks of edges so each SBUF partition holds a
    contiguous run of edge rows per block (multi-KB descriptors -> line-rate
    HBM reads). Scatter-mean becomes dense one-hot matmuls: per block, ONE
    DVE tensor_tensor(is_equal) builds the block's one-hot
    [128e, Jg*64n] bf16 (tiled-iota constant vs per-slot local-target scalars
    via a stride-0 broadcast AP); the PE accumulates
    aggT[f, n] += attr_chunk.T @ onehot_chunk into a per-group PSUM bank.
    64-node blocks halve the DVE one-hot work vs 128-node blocks.
    recip = 1/max(degree,1) is replicated across partitions by a K=1 PE
    matmul (ones.T @ recip_row), copied to SBUF by ACT, and applied by one
    DVE multiply per group while evicting the aggregate PSUM->SBUF.
  - MLP in transposed layout, features on partitions: y1T = relu(W1T.T @
    [xT; aggT] + b1), y2T = W2T.T @ y1T + b2, biases applied by the ACT
    engine at PSUM eviction. MLP operands in bf16 (PSUM accumulation stays
    fp32); final output is fp32. Output stays transposed [128, nodes]; the
    host transposes back while unsharding.
"""

import numpy as np
import ml_dtypes

N_NODES = 100_000
N_EDGES = 1_600_000
F = 128
HIDDEN = 256
OUT_F = 128
N_CORES = 8
P = 128
NODES_PER_CORE = N_NODES // N_CORES          # 12500
NODE_B = 64                                  # nodes per aggregation block
BLOCKS = -(-NODES_PER_CORE // NODE_B)        # 196
GROUP_NODES = 512                            # MLP group width
GROUP_BLOCKS = GROUP_NODES // NODE_B         # 8 blocks per group
N_GROUPS = -(-BLOCKS // GROUP_BLOCKS)        # 25 (last group partial)
NLOC = BLOCKS * NODE_B                       # 12544
MLP_BF16 = True

BF16 = ml_dtypes.bfloat16

_COMPILED = {}
LAST_EXEC_NS = None
LAST_RESULTS = None


def _preprocess(x, edge_index, edge_attr, W1, b1, W2, b2):
    """Host routing: sort edge ids by target node, build per-core contiguous
    group-padded edge buffers + per-slot local-target tables."""
    col = np.asarray(edge_index[1]).astype(np.int64)
    order = np.argsort(col, kind="stable")
    sorted_col = col[order]

    counts = np.bincount(col, minlength=N_NODES)
    recip_full = (1.0 / np.maximum(counts, 1)).astype(np.float32)

    lows = np.empty(N_CORES * BLOCKS, np.int64)
    highs = np.empty(N_CORES * BLOCKS, np.int64)
    for c in range(N_CORES):
        base = c * NODES_PER_CORE
        for b in range(BLOCKS):
            i = c * BLOCKS + b
            lows[i] = base + b * NODE_B
            highs[i] = min(base + (b + 1) * NODE_B, base + NODES_PER_CORE)
    starts = np.searchsorted(sorted_col, lows, side="left")
    ends = np.searchsorted(sorted_col, highs, side="left")
    n_cb = (ends - starts).reshape(N_CORES, BLOCKS)

    # per-GROUP uniform 128-edge chunk count (shared across cores + blocks of
    # the group, so a group's 8 blocks form one rectangular DMA)
    n_max_b = n_cb.max(axis=0)
    Jg = np.zeros(N_GROUPS, np.int64)
    for g in range(N_GROUPS):
        b0, b1_ = g * GROUP_BLOCKS, min((g + 1) * GROUP_BLOCKS, BLOCKS)
        Jg[g] = max(1, int(-(-int(n_max_b[b0:b1_].max()) // P)))
    gnb = [min(GROUP_BLOCKS, BLOCKS - g * GROUP_BLOCKS) for g in range(N_GROUPS)]
    cap_g = Jg * P                                  # rows per block in group g
    rows_g = cap_g * gnb                            # rows per group
    offg = np.zeros(N_GROUPS + 1, np.int64)
    offg[1:] = np.cumsum(rows_g)
    E_pad = int(offg[-1])

    cols_g = Jg * gnb                               # lt cols per group
    cog = np.zeros(N_GROUPS + 1, np.int64)
    cog[1:] = np.cumsum(cols_g)
    TOTC = int(cog[-1])

    ea16 = np.asarray(edge_attr, np.float32).astype(BF16)

    attr = np.zeros((N_CORES, E_pad, F), BF16)
    lt_all = np.full((N_CORES, P, TOTC), 3000.0, BF16)
    si = starts.reshape(N_CORES, BLOCKS)
    for c in range(N_CORES):
        for b in range(BLOCKS):
            g, bl = b // GROUP_BLOCKS, b % GROUP_BLOCKS
            n = int(n_cb[c, b])
            jb = int(Jg[g])
            cap = jb * P
            o = int(offg[g]) + bl * cap
            if n:
                s = int(si[c, b])
                attr[c, o:o + n] = ea16[order[s:s + n]]
                tgt = sorted_col[s:s + n]
                ltb = np.full(cap, 3000.0, np.float32)
                ltb[:n] = (tgt - lows[c * BLOCKS + b]).astype(np.float32)
                # slot r = p*jb + j  ->  [128, jb]
                co = int(cog[g]) + bl * jb
                lt_all[c, :, co:co + jb] = ltb.reshape(P, jb).astype(BF16)

    # per-core recip over padded local nodes
    recip_loc = np.ones((N_CORES, NLOC), np.float32)
    for c in range(N_CORES):
        recip_loc[c, :NODES_PER_CORE] = \
            recip_full[c * NODES_PER_CORE:(c + 1) * NODES_PER_CORE]

    mdt = BF16 if MLP_BF16 else np.float32
    xT = np.zeros((N_CORES, F, NLOC), mdt)
    xt_full = np.ascontiguousarray(np.asarray(x, np.float32).T)
    for c in range(N_CORES):
        xT[c, :, :NODES_PER_CORE] = \
            xt_full[:, c * NODES_PER_CORE:(c + 1) * NODES_PER_CORE].astype(mdt)

    w1t = np.ascontiguousarray(np.asarray(W1, np.float32).T).astype(mdt)
    w2t = np.ascontiguousarray(np.asarray(W2, np.float32).T).astype(mdt)
    Jmax = int(Jg.max())
    iota_big = np.broadcast_to(np.arange(NODE_B, dtype=np.float32),
                               (Jmax, NODE_B)).reshape(1, Jmax * NODE_B)
    iota_big = np.broadcast_to(iota_big, (P, Jmax * NODE_B)).astype(BF16)

    in_maps = []
    for c in range(N_CORES):
        in_maps.append({
            "ea": np.ascontiguousarray(attr[c]),
            "lt": np.ascontiguousarray(lt_all[c]),
            "recip": np.ascontiguousarray(recip_loc[c]),
            "xT": np.ascontiguousarray(xT[c]),
            "w1t": w1t,
            "w2t": w2t,
            "b1": np.asarray(b1, np.float32),
            "b2": np.asarray(b2, np.float32),
            "iotab": np.ascontiguousarray(iota_big),
        })
    params = tuple(int(v) for v in Jg)
    return in_maps, params


def _build(params):
    """Build + compile the per-core Bass program (same NEFF for all cores)."""
    import concourse.bass as bass
    import concourse.bacc as bacc
    import concourse.tile as tile
    import concourse.mybir as mybir

    Jg = list(params)
    f32 = mybir.dt.float32
    bf16 = mybir.dt.bfloat16
    mdt = bf16 if MLP_BF16 else f32
    gnb = [min(GROUP_BLOCKS, BLOCKS - g * GROUP_BLOCKS) for g in range(N_GROUPS)]
    cap_g = [P * j for j in Jg]
    rows_g = [cap_g[g] * gnb[g] for g in range(N_GROUPS)]
    offg = np.concatenate([[0], np.cumsum(rows_g)]).astype(int)
    E_pad = int(offg[-1])
    cols_g = [Jg[g] * gnb[g] for g in range(N_GROUPS)]
    cog = np.concatenate([[0], np.cumsum(cols_g)]).astype(int)
    TOTC = int(cog[-1])
    Jmax = max(Jg)

    nc = bacc.Bacc("TRN2", target_bir_lowering=False, debug=False,
                   num_devices=N_CORES)
    ea_d = nc.dram_tensor("ea", [E_pad, F], bf16, kind="ExternalInput").ap()
    lt_d = nc.dram_tensor("lt", [P, TOTC], bf16, kind="ExternalInput").ap()
    rc_d = nc.dram_tensor("recip", [NLOC], f32, kind="ExternalInput").ap()
    xt_d = nc.dram_tensor("xT", [F, NLOC], mdt, kind="ExternalInput").ap()
    w1t_d = nc.dram_tensor("w1t", [HIDDEN, HIDDEN], mdt, kind="ExternalInput").ap()
    w2t_d = nc.dram_tensor("w2t", [HIDDEN, OUT_F], mdt, kind="ExternalInput").ap()
    b1_d = nc.dram_tensor("b1", [HIDDEN], f32, kind="ExternalInput").ap()
    b2_d = nc.dram_tensor("b2", [OUT_F], f32, kind="ExternalInput").ap()
    io_d = nc.dram_tensor("iotab", [P, Jmax * NODE_B], bf16, kind="ExternalInput").ap()
    out_d = nc.dram_tensor("out", [OUT_F, NLOC], f32, kind="ExternalOutput").ap()

    with tile.TileContext(nc) as tc:
        with (
            tc.tile_pool(name="const", bufs=1) as cp,
            tc.tile_pool(name="tb", bufs=3) as tbp,
            tc.tile_pool(name="ga", bufs=3) as gap,
            tc.tile_pool(name="oh", bufs=6) as ohp,
            tc.tile_pool(name="mlp", bufs=2) as mp,
            tc.tile_pool(name="agg_ps", bufs=2, space="PSUM") as aps,
            tc.tile_pool(name="y1_ps", bufs=2, space="PSUM") as y1ps,
            tc.tile_pool(name="y2_ps", bufs=1, space="PSUM") as y2ps,
        ):
            # ---- constants ----
            iota_t = cp.tile([P, Jmax * NODE_B], bf16)
            nc.scalar.dma_start(out=iota_t[:], in_=io_d[:])
            w1t_t = []
            for fc in range(2):
                w1c = cp.tile([P, HIDDEN], mdt, name=f"w1c{fc}")
                nc.scalar.dma_start(out=w1c[:], in_=w1t_d[fc * P:(fc + 1) * P, :])
                w1t_t.append(w1c)
            w2t_t = []
            for oc in range(2):
                w2c = cp.tile([P, OUT_F], mdt, name=f"w2c{oc}")
                nc.scalar.dma_start(out=w2c[:], in_=w2t_d[oc * P:(oc + 1) * P, :])
                w2t_t.append(w2c)
            b1_t = []
            for oh in range(2):
                b1c = cp.tile([P, 1], f32, name=f"b1c{oh}")
                nc.scalar.dma_start(out=b1c[:], in_=b1_d[oh * P:(oh + 1) * P, None])
                b1_t.append(b1c)
            b2_t = cp.tile([P, 1], f32)
            nc.scalar.dma_start(out=b2_t[:], in_=b2_d[:, None])
            ones_t = cp.tile([1, P], f32)
            nc.vector.memset(ones_t[:], 1.0)
            rcrow_t = cp.tile([1, NLOC], f32)
            nc.scalar.dma_start(out=rcrow_t[:], in_=rc_d[None, :])

            for g in range(N_GROUPS):
                gb0 = g * GROUP_BLOCKS
                nb = gnb[g]
                W = nb * NODE_B
                jb = Jg[g]
                cap = cap_g[g]
                row0 = int(offg[g])
                cg0 = int(cog[g])

                lt_t = tbp.tile([P, nb * jb], bf16, tag="lt")
                nc.scalar.dma_start(out=lt_t[:], in_=lt_d[:, cg0:cg0 + nb * jb])

                # whole group's edges in ONE DMA; partition p holds, per block,
                # the contiguous run [row0 + bl*cap + p*jb, +jb)
                ga_t = gap.tile([P, nb * jb * F], bf16, tag="ga")
                nc.sync.dma_start(
                    out=ga_t[:].rearrange("p (b j f) -> p b j f", b=nb, j=jb),
                    in_=ea_d[row0:row0 + rows_g[g], :].rearrange(
                        "(b p j) f -> p b j f", p=P, j=jb))

                # replicate recip across partitions: PE ones.T @ recip_row
                rr_ps = y2ps.tile([P, W], f32, tag="rrps")
                nc.tensor.matmul(out=rr_ps[:], lhsT=ones_t[:],
                                 rhs=rcrow_t[:, gb0 * NODE_B:gb0 * NODE_B + W],
                                 start=True, stop=True)
                rr_t = mp.tile([P, W], f32, tag="rr")
                nc.scalar.copy(out=rr_t[:], in_=rr_ps[:])

                agg_ps = aps.tile([P, W], f32, tag="agg")
                for bl in range(nb):
                    cb0 = bl * jb
                    # one-hot for the whole block in ONE DVE op:
                    # oh[p, j, n] = (iota[n] == lt[p, cb0+j])
                    oh_t = ohp.tile([P, jb * NODE_B], bf16, tag="oh")
                    nc.vector.tensor_tensor(
                        out=oh_t[:],
                        in0=iota_t[:, :jb * NODE_B],
                        in1=lt_t[:, cb0:cb0 + jb, None].to_broadcast(
                            [P, jb, NODE_B]),
                        op=mybir.AluOpType.is_equal)
                    for i in range(jb):
                        nc.tensor.matmul(
                            out=agg_ps[:, bl * NODE_B:(bl + 1) * NODE_B],
                            lhsT=ga_t[:, (bl * jb + i) * P:(bl * jb + i + 1) * P],
                            rhs=oh_t[:, i * NODE_B:(i + 1) * NODE_B],
                            start=(i == 0), stop=(i == jb - 1))

                # scale by recip while evicting PSUM -> SBUF (one DVE op)
                aggT_sb = mp.tile([P, W], mdt, tag="aggT")
                nc.vector.tensor_tensor(
                    out=aggT_sb[:], in0=agg_ps[:], in1=rr_t[:],
                    op=mybir.AluOpType.mult)

                # ---- MLP over this group's W nodes (transposed layout) ----
                xt_sb = mp.tile([P, W], mdt, tag="xt")
                nc.scalar.dma_start(out=xt_sb[:],
                                    in_=xt_d[:, gb0 * NODE_B:gb0 * NODE_B + W])

                y1_sb = []
                for oh in range(2):
                    y1_ps = y1ps.tile([P, W], f32, tag=f"y1_{oh}")
                    nc.tensor.matmul(out=y1_ps[:], lhsT=w1t_t[0][:, oh * P:(oh + 1) * P],
                                     rhs=xt_sb[:], start=True, stop=False)
                    nc.tensor.matmul(out=y1_ps[:], lhsT=w1t_t[1][:, oh * P:(oh + 1) * P],
                                     rhs=aggT_sb[:], start=False, stop=True)
                    y1c = mp.tile([P, W], mdt, tag=f"y1sb{oh}", name=f"y1c{oh}")
                    nc.scalar.activation(out=y1c[:], in_=y1_ps[:],
                                         func=mybir.ActivationFunctionType.Relu,
                                         bias=b1_t[oh][:])
                    y1_sb.append(y1c)

                y2_ps = y2ps.tile([P, W], f32, tag="y2")
                nc.tensor.matmul(out=y2_ps[:], lhsT=w2t_t[0][:], rhs=y1_sb[0][:],
                                 start=True, stop=False)
                nc.tensor.matmul(out=y2_ps[:], lhsT=w2t_t[1][:], rhs=y1_sb[1][:],
                                 start=False, stop=True)
                y2_sb = mp.tile([P, W], f32, tag="y2sb")
                nc.scalar.activation(out=y2_sb[:], in_=y2_ps[:],
                                     func=mybir.ActivationFunctionType.Identity,
                                     bias=b2_t[:])
                nc.scalar.dma_start(out=out_d[:, gb0 * NODE_B:gb0 * NODE_B + W],
                                    in_=y2_sb[:])

    nc.compile()
    return nc


def kernel(x, edge_index, edge_attr, W1, b1, W2, b2, _trace=False):
    global LAST_EXEC_NS, LAST_RESULTS
    from concourse.bass_utils import run_bass_kernel_spmd

    in_maps, params = _preprocess(x, edge_index, edge_attr, W1, b1, W2, b2)
    if params not in _COMPILED:
        _COMPILED[params] = _build(params)
    nc = _COMPILED[params]

    res = run_bass_kernel_spmd(nc, in_maps, core_ids=list(range(N_CORES)),
                               trace=_trace)
    LAST_EXEC_NS = res.exec_time_ns
    LAST_RESULTS = res
    out = np.empty((N_NODES, OUT_F), np.float32)
    for c, r in enumerate(res.results):
        out[c * NODES_PER_CORE:(c + 1) * NODES_PER_CORE] = \
            r["out"][:, :NODES_PER_CORE].T
    return out
